# revision 7
# baseline (speedup 1.0000x reference)
"""Trainium2 Bass kernel for the Koopman-operator rollout.

Reference computation: y0 = x[:, 0, :]  (shape [2048, 256]);
    y_t = y_{t-1} @ W.T  for t = 1..512, Y[:, t-1, :] = y_t.
Output: [2048, 512, 256] fp32 (1 GiB) -> memory-bound target.

Strategy (8 cores, data-parallel over batch, 256 rows/core):
  Let Wt = W.T.  Y[:, t] = y0 @ Wt^{t+1}.
  * Precompute P_j = Wt^j for j=1..16 via a log-depth product tree.
    Products use the duality Q_j = W^j = (P_j)^T so every product is
    expressible as matmul(out = lhsT.T @ rhs) with natural layouts.
  * Checkpoint states Z_i = y0 @ Wt^{16 i} (i=0..31), kept TRANSPOSED
    (k on partitions) so they can serve as matmul operands. Computed by
    prefix-doubling jumps A_m = Wt^{16 m} (m=1,2,4,8,16) -> rounding
    depth O(log T) instead of 512.
  * Per checkpoint i: Y[:, 16i+j-1] = Z_i @ P_j for j=1..16, as dense
    N=512 matmuls with Z_i^T stationary; PSUM -> SBUF copies on
    DVE/ACT; 2 MiB HWDGE DMAs to HBM.
  Matmul-operand tiles are allocated as float32r (full PE rate at
  N>=256, reduced multiply precision, fp32 PSUM accumulation); the
  PSUM->SBUF copies perform the required f32->f32r rounding.

  Cost-model timeline: ~402 us/core, vs a ~374 us HBM-write floor for
  the 128 MiB/core output (measured rel err vs fp32 CPU oracle: 2.2e-3).
"""

import os

import numpy as np

import concourse.bass as bass
import concourse.mybir as mybir
import concourse.tile as tile
from concourse import bacc
from concourse.bass import ds
from concourse.bass_utils import run_bass_kernel_spmd
from concourse.masks import make_identity

F32 = mybir.dt.float32
F32R = mybir.dt.float32r
BF16 = mybir.dt.bfloat16

# Output HBM format: bf16 halves the dominant HBM write traffic (128 MiB ->
# 64 MiB per core); the fp32 upconvert happens on the host after gather.
# Quantization adds ~1e-3 rel err, far under the 2e-2 gate.
OUT_DT = BF16

N_CORES = 8
B_FULL = 2048
B_SH = B_FULL // N_CORES  # 256 batch rows per core
K = 256  # state dim
T = 512  # time steps
S = 16  # timesteps per checkpoint chunk
M = T // S  # 32 checkpoints

# engine split for PSUM->SBUF output copies: of every K_COPY_MOD tiles,
# the first K_COPY_DVE go to VectorE (DVE), the rest to ScalarE (ACT).
# DVE also carries the P-tree/Z-jump copies, so it gets the minority share.
COPY_DVE = int(os.environ.get("K_COPY_DVE", "2"))
COPY_MOD = int(os.environ.get("K_COPY_MOD", "5"))


def _mm(nc, out, lhsT, rhs, start, stop):
    # operands are float32r tiles already (producers round to f32r)
    nc.tensor.matmul(out, lhsT, rhs, start=start, stop=stop)


class _Mat:
    """A 256x256 matrix stored as an SBUF tile [128, 2, 256]:
    elem (p, h, c) = M[h*128 + p, c]."""

    def __init__(self, ap):
        self.ap = ap

    def half(self, hm):
        # [128, 256] slice: rows hm*128 .. hm*128+127 (partition = row)
        return self.ap[:, hm, :]

    def blk(self, hm, hc):
        # [128, 128] block: rows hm*128.., cols hc*128..
        return self.ap[:, hm, ds(128 * hc, 128)]


def _product(nc, psum_pool, dst, lhsT_mat, rhs_mat):
    """dst = lhsT_mat.T @ rhs_mat  (all 256x256 _Mats)."""
    for ha in range(2):
        ps = psum_pool.tile([128, 256], F32, tag="psz", name=f"psz_{ha}")
        for hm in range(2):
            _mm(nc, ps, lhsT_mat.blk(hm, ha), rhs_mat.half(hm), hm == 0, hm == 1)
        nc.vector.tensor_copy(dst.half(ha), ps)


def _build_program(act_every=None, ladder_late=False, dma_alt=False):
    global ACT_COPY_EVERY
    if act_every is not None:
        ACT_COPY_EVERY = act_every
    nc = bacc.Bacc(
        "TRN2",
        target_bir_lowering=False,
        debug=False,
        enable_asserts=False,
        num_devices=N_CORES,
    )
    x_d = nc.dram_tensor("x", [B_SH, K], F32, kind="ExternalInput").ap()
    w_d = nc.dram_tensor("w", [K, K], F32, kind="ExternalInput").ap()
    y_d = nc.dram_tensor("y", [B_SH, T, K], OUT_DT, kind="ExternalOutput").ap()

    with tile.TileContext(nc) as tc:
        with (
            tc.tile_pool(name="consts", bufs=1) as consts,
            tc.tile_pool(name="mats", bufs=1) as mats,
            tc.tile_pool(name="zts", bufs=1) as zts,
            tc.tile_pool(name="ostage", bufs=int(os.environ.get("K_OST", "3"))) as ostage,
            tc.tile_pool(name="pso", bufs=int(os.environ.get("K_PSO", "5")), space="PSUM") as pso,
            tc.tile_pool(name="psz", bufs=int(os.environ.get("K_PSZ", "3")), space="PSUM") as psz,
        ):
            ident = consts.tile([128, 128], F32, tag="ident", name="ident")
            make_identity(nc, ident)

            w_nat = consts.tile([128, 2, K], F32, tag="w_nat", name="w_nat")
            x_nat = consts.tile([128, 2, K], F32, tag="x_nat", name="x_nat")
            for h in range(2):
                nc.sync.dma_start(out=w_nat[:, h, :], in_=w_d[ds(128 * h, 128), :])
                nc.sync.dma_start(out=x_nat[:, h, :], in_=x_d[ds(128 * h, 128), :])

            # Pcat holds P_1..P_16 row-half-major: [128, 2, 16*256]
            pcat = mats.tile([128, 2, S * K], F32R, tag="pcat", name="pcat")

            def P(j):  # 1-indexed power as a _Mat-like view
                class V:
                    def half(self, hm, _j=j):
                        return pcat[:, hm, ds(K * (_j - 1), K)]

                    def blk(self, hm, hc, _j=j):
                        return pcat[:, hm, ds(K * (_j - 1) + 128 * hc, 128)]

                return V()

            w_r = consts.tile([128, 2, K], F32R, tag="w_r", name="w_r")
            for h in range(2):
                nc.vector.tensor_copy(w_r[:, h, :], w_nat[:, h, :])
            q1 = _Mat(w_r)  # Q_1 = W (natural layout, rounded to f32r)

            # --- transposes: Z0^T = x^T, P_1 = W^T (PE transpose via identity)
            zt = [None] * M
            zt[0] = _Mat(zts.tile([128, 2, K], F32R, tag="zt0", name="zt0"))
            p1 = P(1)
            # W-transposes first: P_1 gates the whole P-tree; Z0^T is not
            # needed until the first output matmuls.
            for g in range(2):
                for h in range(2):
                    pst2 = psz.tile([128, 128], F32, tag="psz", name=f"pstw_{g}_{h}")
                    nc.tensor.transpose(pst2, w_nat[:, g, ds(128 * h, 128)], ident)
                    nc.vector.tensor_copy(pcat[:, h, ds(128 * g, 128)], pst2)
            for g in range(2):
                for h in range(2):
                    pst = psz.tile([128, 128], F32, tag="psz", name=f"pst_{g}_{h}")
                    nc.tensor.transpose(pst, x_nat[:, g, ds(128 * h, 128)], ident)
                    nc.vector.tensor_copy(zt[0].ap[:, h, ds(128 * g, 128)], pst)

            # --- P-tree: P_1..P_16 (+ Q_2, Q_4, Q_8)
            def mk(tag):
                return _Mat(mats.tile([128, 2, K], F32R, tag=tag, name=tag))

            # --- checkpoint Z-tree (prefix doubling) interleaved with outputs
            copy_ctr = [0]

            merge_dma = bool(int(os.environ.get("K_MERGE_DMA", "0")))
            y_r = y_d.rearrange("(h p) t k -> p h t k", p=128)

            def emit_outputs_merged(i):
                ost = ostage.tile(
                    [128, 2, S, K], OUT_DT, tag="ostm", bufs=2, name=f"ostm_{i}"
                )
                for m in range(2):
                    pos = {}
                    for n in range(8):
                        pos[n] = pso.tile(
                            [128, 2, K], F32, tag="pso", name=f"pso_{i}_{m}_{n}"
                        )
                    for hm in range(2):
                        lhsT = zt[i].ap[:, hm, ds(128 * m, 128)]
                        for n in range(8):
                            rhs = pcat[:, hm, ds(512 * n, 512)]
                            _mm(nc, pos[n], lhsT, rhs, hm == 0, hm == 1)
                    for n in range(8):
                        dst = ost[:, m, ds(2 * n, 2), :]
                        if copy_ctr[0] % ACT_COPY_EVERY == ACT_COPY_EVERY - 1:
                            nc.scalar.copy(dst, pos[n])
                        else:
                            nc.vector.tensor_copy(dst, pos[n])
                        copy_ctr[0] += 1
                nc.sync.dma_start(
                    out=y_r[:, :, ds(S * i, S), :], in_=ost
                )

            def emit_outputs(i, ns=range(8), dma_split=1, ms=(0, 1)):
                if merge_dma and len(list(ns)) == 8:
                    return emit_outputs_merged(i)
                """Y[:, 16i + j - 1, :] = Z_i @ P_j for j-pairs in ns.
                dma_split: number of DMAs the staged chunk is divided into
                (must divide len(ns); chunks are contiguous j-pair groups)."""
                ns = list(ns)
                for m in ms:  # batch half
                    ost = ostage.tile(
                        [128, S, K], OUT_DT, tag="ost", name=f"ost_{i}_{m}"
                    )
                    pos = {}
                    for n in ns:
                        pos[n] = pso.tile(
                            [128, 2, K], F32, tag="pso", name=f"pso_{i}_{m}_{n}"
                        )
                    for hm in range(2):
                        lhsT = zt[i].ap[:, hm, ds(128 * m, 128)]
                        for n in ns:
                            # rhs: P_{2n+1}, P_{2n+2} concatenated = 512 cols
                            rhs = pcat[:, hm, ds(512 * n, 512)]
                            _mm(nc, pos[n], lhsT, rhs, hm == 0, hm == 1)
                    per_dma = len(ns) // dma_split
                    for g in range(dma_split):
                        grp = ns[g * per_dma : (g + 1) * per_dma]
                        for n in grp:
                            dst = ost[:, ds(2 * n, 2), :]
                            if copy_ctr[0] % ACT_COPY_EVERY == ACT_COPY_EVERY - 1:
                                nc.scalar.copy(dst, pos[n])
                            else:
                                nc.vector.tensor_copy(dst, pos[n])
                            copy_ctr[0] += 1
                        dma_eng = (
                            nc.scalar if (dma_alt and (i + m) % 2 == 1) else nc.sync
                        )
                        n0 = grp[0]
                        dma_eng.dma_start(
                            out=y_d[
                                ds(128 * m, 128),
                                ds(S * i + 2 * n0, 2 * len(grp)),
                                :,
                            ],
                            in_=ost[:, ds(2 * n0, 2 * len(grp)), :],
                        )

            def emit_zjump(dst_i, src_i, m):
                zt[dst_i] = _Mat(
                    zts.tile([128, 2, K], F32R, tag=f"zt{dst_i}", name=f"zt{dst_i}")
                )
                # Z_{dst}^T = A_m^T @ Z_{src}^T
                _product(nc, psz, zt[dst_i], amat[m], zt[src_i])

            q2, q4, q8 = mk("q2"), mk("q4"), mk("q8")
            _product(nc, psz, P(2), q1, p1)  # P2 = Q1.T @ P1 = Wt^2
            _product(nc, psz, q2, p1, q1)  # Q2 = P1.T @ Q1 = W^2
            _product(nc, psz, P(3), q1, P(2))
            _product(nc, psz, P(4), q2, P(2))
            out0_early = bool(int(os.environ.get("K_OUT0_EARLY", "0")))
            if out0_early:
                emit_outputs(0, ns=[0, 1], dma_split=1)  # needs P1..P4 only
            _product(nc, psz, q4, P(2), q2)
            for j in range(1, 5):
                _product(nc, psz, P(4 + j), q4, P(j))
            if out0_early:
                emit_outputs(0, ns=[2, 3], dma_split=1)  # needs P5..P8
            _product(nc, psz, q8, P(4), q4)
            for j in range(1, 9):
                _product(nc, psz, P(8 + j), q8, P(j))

            # --- A-ladder: A_m = Wt^{16 m} for m=1,2,4,8,16 (A_1 = P_16)
            def emit_ladder():
                q16 = mk("q16")
                _product(nc, psz, q16, P(8), q8)  # W^16
                a2, a4, a8, a16 = mk("a2"), mk("a4"), mk("a8"), mk("a16")
                qlad_a = mk("qlad_a")  # Q32, then Q128
                qlad_b = mk("qlad_b")  # Q64
                a1 = P(16)
                _product(nc, psz, a2, q16, a1)  # Wt^32
                _product(nc, psz, qlad_a, a1, q16)  # W^32
                _product(nc, psz, a4, qlad_a, a2)  # Wt^64
                _product(nc, psz, qlad_b, a2, qlad_a)  # W^64
                _product(nc, psz, a8, qlad_b, a4)  # Wt^128
                _product(nc, psz, qlad_a, a4, qlad_b)  # W^128 (reuse slot)
                _product(nc, psz, a16, qlad_a, a8)  # Wt^256
                return {1: a1, 2: a2, 4: a4, 8: a8, 16: a16}

            amat = None if ladder_late else emit_ladder()

            lad_mid = bool(int(os.environ.get("K_LAD_MID", "0")))
            if out0_early:
                emit_outputs(0, ns=[4, 5, 6, 7], dma_split=2)
            elif lad_mid:
                emit_outputs(0, ms=(0,))
            else:
                emit_outputs(0)
            if ladder_late or lad_mid:
                amat = emit_ladder()
            if lad_mid and not out0_early:
                emit_outputs(0, ms=(1,))
            emit_zjump(16, 0, 16)
            emit_outputs(16)
            emit_zjump(8, 0, 8)
            emit_zjump(24, 16, 8)
            emit_outputs(8)
            emit_outputs(24)
            for src in (0, 8, 16, 24):
                emit_zjump(src + 4, src, 4)
            for src in (4, 12, 20, 28):
                emit_outputs(src)
            for src in (0, 4, 8, 12, 16, 20, 24, 28):
                emit_zjump(src + 2, src, 2)
            for src in (2, 6, 10, 14, 18, 22, 26, 30):
                emit_outputs(src)
            if int(os.environ.get("K_ODD_PAIR", "0")):
                for src in range(0, 31, 2):
                    emit_zjump(src + 1, src, 1)
                    emit_outputs(src + 1)
            else:
                for src in range(0, 31, 2):
                    emit_zjump(src + 1, src, 1)
                for src in range(1, 32, 2):
                    emit_outputs(src)

    nc.compile()
    return nc


_cached_nc = None
_last_results = None


def kernel(x, W, T=None):
    global _cached_nc, _last_results
    if _cached_nc is None:
        _cached_nc = _build_program()
    nc = _cached_nc

    x2 = np.ascontiguousarray(np.asarray(x, dtype=np.float32).reshape(B_FULL, K))
    w2 = np.ascontiguousarray(np.asarray(W, dtype=np.float32))
    in_maps = [
        {"x": x2[i * B_SH : (i + 1) * B_SH], "w": w2} for i in range(N_CORES)
    ]
    res = run_bass_kernel_spmd(
        nc,
        in_maps,
        core_ids=list(range(N_CORES)),
        trace=bool(os.environ.get("BASS_TRACE")),
    )
    _last_results = res
    y = np.concatenate(
        [np.asarray(res.results[i]["y"]).astype(np.float32) for i in range(N_CORES)],
        axis=0,
    )
    return y



# revision 19
# speedup vs baseline: 1.5066x; 1.5066x over previous
"""Trainium2 Bass kernel for the Koopman-operator rollout.

Reference computation: y0 = x[:, 0, :]  (shape [2048, 256]);
    y_t = y_{t-1} @ W.T  for t = 1..512, Y[:, t-1, :] = y_t.
Output: [2048, 512, 256] fp32 (1 GiB) -> memory-bound target.

Strategy (8 cores, data-parallel over batch, 256 rows/core):
  Let Wt = W.T.  Y[:, t] = y0 @ Wt^{t+1}.
  * Precompute P_j = Wt^j for j=1..16 via a log-depth product tree.
    Products use the duality Q_j = W^j = (P_j)^T so every product is
    expressible as matmul(out = lhsT.T @ rhs) with natural layouts.
  * Checkpoint states Z_i = y0 @ Wt^{16 i} (i=0..31), kept TRANSPOSED
    (k on partitions) so they can serve as matmul operands. Computed by
    prefix-doubling jumps A_m = Wt^{16 m} (m=1,2,4,8,16) -> rounding
    depth O(log T) instead of 512.
  * Per checkpoint i: Y[:, 16i+j-1] = Z_i @ P_j for j=1..16, as dense
    N=512 matmuls with Z_i^T stationary; PSUM -> SBUF copies on
    DVE/ACT; 2 MiB HWDGE DMAs to HBM.
  Matmul-operand tiles are allocated as float32r (full PE rate at
  N>=256, reduced multiply precision, fp32 PSUM accumulation); the
  PSUM->SBUF copies perform the required f32->f32r rounding.

  Cost-model timeline: ~402 us/core, vs a ~374 us HBM-write floor for
  the 128 MiB/core output (measured rel err vs fp32 CPU oracle: 2.2e-3).
"""

import os

import numpy as np

import concourse.bass as bass
import concourse.mybir as mybir
import concourse.tile as tile
from concourse import bacc
from concourse.bass import ds
from concourse.bass_utils import run_bass_kernel_spmd
from concourse.masks import make_identity

F32 = mybir.dt.float32
F32R = mybir.dt.float32r
BF16 = mybir.dt.bfloat16

# Output HBM format: bf16 halves the dominant HBM write traffic (128 MiB ->
# 64 MiB per core); the fp32 upconvert happens on the host after gather.
# Quantization adds ~1e-3 rel err, far under the 2e-2 gate.
OUT_DT = BF16

N_CORES = 8
B_FULL = 2048
B_SH = B_FULL // N_CORES  # 256 batch rows per core
K = 256  # state dim
T = 512  # time steps
S = 16  # timesteps per checkpoint chunk
M = T // S  # 32 checkpoints

# engine split for PSUM->SBUF output copies: of every K_COPY_MOD tiles,
# the first K_COPY_DVE go to VectorE (DVE), the rest to ScalarE (ACT).
# DVE also carries the P-tree/Z-jump copies, so it gets the minority share.
COPY_DVE = int(os.environ.get("K_COPY_DVE", "1"))
COPY_MOD = int(os.environ.get("K_COPY_MOD", "2"))


def _mm(nc, out, lhsT, rhs, start, stop):
    # operands are float32r tiles already (producers round to f32r)
    nc.tensor.matmul(out, lhsT, rhs, start=start, stop=stop)


class _Mat:
    """A 256x256 matrix stored as an SBUF tile [128, 2, 256]:
    elem (p, h, c) = M[h*128 + p, c]."""

    def __init__(self, ap):
        self.ap = ap

    def half(self, hm):
        # [128, 256] slice: rows hm*128 .. hm*128+127 (partition = row)
        return self.ap[:, hm, :]

    def blk(self, hm, hc):
        # [128, 128] block: rows hm*128.., cols hc*128..
        return self.ap[:, hm, ds(128 * hc, 128)]


_prod_ctr = [0]


def _product(nc, psum_pool, dst, lhsT_mat, rhs_mat):
    """dst = lhsT_mat.T @ rhs_mat  (all 256x256 _Mats).

    One full-bank PSUM tile + a single [128, 512] copy, alternating the
    copy engine per product so chained products don't serialize on DVE."""
    ps = psum_pool.tile([128, 2, 256], F32, tag="psz", name=f"psz_{_prod_ctr[0]}")
    for ha in range(2):
        for hm in range(2):
            _mm(nc, ps[:, ha, :], lhsT_mat.blk(hm, ha), rhs_mat.half(hm),
                hm == 0, hm == 1)
    if _prod_ctr[0] % 2 == 0:
        nc.vector.tensor_copy(dst.ap, ps)
    else:
        nc.scalar.copy(dst.ap, ps)
    _prod_ctr[0] += 1


def _build_program(ladder_late=False, dma_alt=False):
    nc = bacc.Bacc(
        "TRN2",
        target_bir_lowering=False,
        debug=False,
        enable_asserts=False,
        num_devices=N_CORES,
    )
    x_d = nc.dram_tensor("x", [B_SH, K], F32, kind="ExternalInput").ap()
    w_d = nc.dram_tensor("w", [K, K], F32, kind="ExternalInput").ap()
    y_d = nc.dram_tensor("y", [B_SH, T, K], OUT_DT, kind="ExternalOutput").ap()

    with tile.TileContext(nc) as tc:
        with (
            tc.tile_pool(name="consts", bufs=1) as consts,
            tc.tile_pool(name="mats", bufs=1) as mats,
            tc.tile_pool(name="zts", bufs=1) as zts,
            tc.tile_pool(name="ostage", bufs=int(os.environ.get("K_OST", "3"))) as ostage,
            tc.tile_pool(name="pso", bufs=int(os.environ.get("K_PSO", "5")), space="PSUM") as pso,
            tc.tile_pool(name="psz", bufs=int(os.environ.get("K_PSZ", "3")), space="PSUM") as psz,
        ):
            ident = consts.tile([128, 128], F32, tag="ident", name="ident")
            make_identity(nc, ident)

            w_nat = consts.tile([128, 2, K], F32, tag="w_nat", name="w_nat")
            x_nat = consts.tile([128, 2, K], F32, tag="x_nat", name="x_nat")
            nc.sync.dma_start(
                out=w_nat, in_=w_d.rearrange("(h p) k -> p h k", p=128)
            )
            nc.sync.dma_start(
                out=x_nat, in_=x_d.rearrange("(h p) k -> p h k", p=128)
            )

            # Pcat holds P_1..P_16 row-half-major: [128, 2, 16*256]
            pcat = mats.tile([128, 2, S * K], F32R, tag="pcat", name="pcat")

            def P(j):  # 1-indexed power as a _Mat-like view
                class V:
                    ap = pcat[:, :, ds(K * (j - 1), K)]

                    def half(self, hm, _j=j):
                        return pcat[:, hm, ds(K * (_j - 1), K)]

                    def blk(self, hm, hc, _j=j):
                        return pcat[:, hm, ds(K * (_j - 1) + 128 * hc, 128)]

                return V()

            w_r = consts.tile([128, 2, K], F32R, tag="w_r", name="w_r")
            for h in range(2):
                nc.vector.tensor_copy(w_r[:, h, :], w_nat[:, h, :])
            q1 = _Mat(w_r)  # Q_1 = W (natural layout, rounded to f32r)

            # --- transposes: Z0^T = x^T, P_1 = W^T (PE transpose via identity)
            zt = [None] * M
            zt[0] = _Mat(zts.tile([128, 2, K], F32R, tag="zt0", name="zt0"))
            p1 = P(1)
            # W-transposes first: P_1 gates the whole P-tree; Z0^T is not
            # needed until the first output matmuls. Both g-blocks of a row
            # half share one PSUM tile -> one [128, 256] copy each.
            for h in range(2):
                tp = psz.tile([128, 2, 128], F32, tag="psz", name=f"pstw_{h}")
                for g in range(2):
                    nc.tensor.transpose(
                        tp[:, g, :], w_nat[:, g, ds(128 * h, 128)], ident
                    )
                eng = nc.vector.tensor_copy if h == 0 else nc.scalar.copy
                eng(pcat[:, h, ds(0, 256)], tp)
            for h in range(2):
                tp = psz.tile([128, 2, 128], F32, tag="psz", name=f"pstx_{h}")
                for g in range(2):
                    nc.tensor.transpose(
                        tp[:, g, :], x_nat[:, g, ds(128 * h, 128)], ident
                    )
                eng = nc.vector.tensor_copy if h == 0 else nc.scalar.copy
                eng(zt[0].ap[:, h, :], tp)

            # --- P-tree: P_1..P_16 (+ Q_2, Q_4, Q_8)
            def mk(tag):
                return _Mat(mats.tile([128, 2, K], F32R, tag=tag, name=tag))

            # --- checkpoint Z-tree (prefix doubling) interleaved with outputs
            copy_ctr = [0]

            merge_dma = bool(int(os.environ.get("K_MERGE_DMA", "0")))
            y_r = y_d.rearrange("(h p) t k -> p h t k", p=128)

            def emit_outputs_merged(i):
                ost = ostage.tile(
                    [128, 2, S, K], OUT_DT, tag="ostm", bufs=2, name=f"ostm_{i}"
                )
                for m in range(2):
                    pos = {}
                    for n in range(8):
                        pos[n] = pso.tile(
                            [128, 2, K], F32, tag="pso", name=f"pso_{i}_{m}_{n}"
                        )
                    for hm in range(2):
                        lhsT = zt[i].ap[:, hm, ds(128 * m, 128)]
                        for n in range(8):
                            rhs = pcat[:, hm, ds(512 * n, 512)]
                            _mm(nc, pos[n], lhsT, rhs, hm == 0, hm == 1)
                    for n in range(8):
                        dst = ost[:, m, ds(2 * n, 2), :]
                        if copy_ctr[0] % COPY_MOD < COPY_DVE:
                            nc.vector.tensor_copy(dst, pos[n])
                        else:
                            nc.scalar.copy(dst, pos[n])
                        copy_ctr[0] += 1
                nc.sync.dma_start(
                    out=y_r[:, :, ds(S * i, S), :], in_=ost
                )

            def emit_outputs(i, ns=range(8), dma_split=1, ms=(0, 1)):
                if merge_dma and len(list(ns)) == 8:
                    return emit_outputs_merged(i)
                """Y[:, 16i + j - 1, :] = Z_i @ P_j for j-pairs in ns.
                dma_split: number of DMAs the staged chunk is divided into
                (must divide len(ns); chunks are contiguous j-pair groups)."""
                ns = list(ns)
                for m in ms:  # batch half
                    ost = ostage.tile(
                        [128, S, K], OUT_DT, tag="ost", name=f"ost_{i}_{m}"
                    )
                    pos = {}
                    for n in ns:
                        pos[n] = pso.tile(
                            [128, 2, K], F32, tag="pso", name=f"pso_{i}_{m}_{n}"
                        )
                    for hm in range(2):
                        lhsT = zt[i].ap[:, hm, ds(128 * m, 128)]
                        for n in ns:
                            # rhs: P_{2n+1}, P_{2n+2} concatenated = 512 cols
                            rhs = pcat[:, hm, ds(512 * n, 512)]
                            _mm(nc, pos[n], lhsT, rhs, hm == 0, hm == 1)
                    per_dma = len(ns) // dma_split
                    for g in range(dma_split):
                        grp = ns[g * per_dma : (g + 1) * per_dma]
                        for n in grp:
                            dst = ost[:, ds(2 * n, 2), :]
                            if copy_ctr[0] % COPY_MOD < COPY_DVE:
                                nc.vector.tensor_copy(dst, pos[n])
                            else:
                                nc.scalar.copy(dst, pos[n])
                            copy_ctr[0] += 1
                        dma_eng = (
                            nc.scalar if (dma_alt and (i + m) % 2 == 1) else nc.sync
                        )
                        n0 = grp[0]
                        dma_eng.dma_start(
                            out=y_d[
                                ds(128 * m, 128),
                                ds(S * i + 2 * n0, 2 * len(grp)),
                                :,
                            ],
                            in_=ost[:, ds(2 * n0, 2 * len(grp)), :],
                        )

            def emit_zjump(dst_i, src_i, m):
                zt[dst_i] = _Mat(
                    zts.tile([128, 2, K], F32R, tag=f"zt{dst_i}", name=f"zt{dst_i}")
                )
                # Z_{dst}^T = A_m^T @ Z_{src}^T
                _product(nc, psz, zt[dst_i], amat[m], zt[src_i])

            q2, q4, q8 = mk("q2"), mk("q4"), mk("q8")
            _product(nc, psz, P(2), q1, p1)  # P2 = Q1.T @ P1 = Wt^2
            _product(nc, psz, q2, p1, q1)  # Q2 = P1.T @ Q1 = W^2
            _product(nc, psz, P(3), q1, P(2))
            _product(nc, psz, P(4), q2, P(2))
            out0_early = bool(int(os.environ.get("K_OUT0_EARLY", "1")))
            if out0_early:
                emit_outputs(0, ns=[0, 1], dma_split=1)  # needs P1..P4 only
            _product(nc, psz, q4, P(2), q2)
            for j in range(1, 5):
                _product(nc, psz, P(4 + j), q4, P(j))
            if out0_early:
                emit_outputs(0, ns=[2, 3], dma_split=1)  # needs P5..P8
            _product(nc, psz, q8, P(4), q4)
            for j in range(1, 9):
                _product(nc, psz, P(8 + j), q8, P(j))

            # --- A-ladder interleaved with anchor jumps + output bursts.
            # A_m = Wt^{16 m}; each ladder product immediately enables the
            # anchor checkpoint 2^k, whose 32 output matmuls (6.8us) hide the
            # next serial ladder step (product+copy ~1.3us). Jump depth stays
            # logarithmic (zt[31] is 5 jumps from zt[0]).
            amat = {1: P(16)}
            q16 = mk("q16")
            _product(nc, psz, q16, P(8), q8)  # W^16
            if out0_early:
                emit_outputs(0, ns=[4, 5], dma_split=1)
            # Each bundle below holds mutually-independent products whose
            # inputs were copied >=1 output burst ago; the following burst
            # (6.8us of PE work) hides their PSUM->SBUF copy latency.
            a2 = mk("a2")
            qlad_a = mk("qlad_a")  # W^32, later reused for W^128
            _product(nc, psz, a2, q16, amat[1])  # Wt^32
            _product(nc, psz, qlad_a, amat[1], q16)  # W^32
            amat[2] = a2
            if out0_early:
                emit_outputs(0, ns=[6, 7], dma_split=1)
            else:
                emit_outputs(0)
            a4 = mk("a4")
            qlad_b = mk("qlad_b")
            emit_zjump(2, 0, 2)
            _product(nc, psz, a4, qlad_a, a2)  # Wt^64
            _product(nc, psz, qlad_b, a2, qlad_a)  # W^64
            amat[4] = a4
            emit_outputs(2)
            a8 = mk("a8")
            emit_zjump(4, 0, 4)
            _product(nc, psz, a8, qlad_b, a4)  # Wt^128
            _product(nc, psz, qlad_a, a4, qlad_b)  # W^128 (reuse slot)
            amat[8] = a8
            emit_outputs(4)
            a16 = mk("a16")
            emit_zjump(8, 0, 8)
            _product(nc, psz, a16, qlad_a, a8)  # Wt^256
            amat[16] = a16
            emit_outputs(8)
            emit_zjump(16, 0, 16)
            emit_outputs(16)
            # Pair every remaining Z-jump with its output burst: the jump's
            # 4 matmuls + 1 copy hide under the following 32 output matmuls,
            # so PSUM/copy backpressure never stalls PE.
            emit_zjump(24, 16, 8)
            emit_outputs(24)
            for src in (8, 16, 24):
                emit_zjump(src + 4, src, 4)
                emit_outputs(src + 4)
            for src in (4, 8, 12, 16, 20, 24, 28):
                emit_zjump(src + 2, src, 2)
                emit_outputs(src + 2)
            last_split = int(os.environ.get("K_LAST_SPLIT", "4"))
            for src in range(0, 31, 2):
                emit_zjump(src + 1, src, 1)
                emit_outputs(
                    src + 1, dma_split=last_split if src == 30 else 1
                )

    nc.compile()
    return nc


_cached_nc = None
_last_results = None


def kernel(x, W, T=None):
    global _cached_nc, _last_results
    if _cached_nc is None:
        _cached_nc = _build_program()
    nc = _cached_nc

    x2 = np.ascontiguousarray(np.asarray(x, dtype=np.float32).reshape(B_FULL, K))
    w2 = np.ascontiguousarray(np.asarray(W, dtype=np.float32))
    in_maps = [
        {"x": x2[i * B_SH : (i + 1) * B_SH], "w": w2} for i in range(N_CORES)
    ]
    res = run_bass_kernel_spmd(
        nc,
        in_maps,
        core_ids=list(range(N_CORES)),
        trace=bool(os.environ.get("BASS_TRACE")),
    )
    _last_results = res
    y = np.concatenate(
        [np.asarray(res.results[i]["y"]).astype(np.float32) for i in range(N_CORES)],
        axis=0,
    )
    return y



# revision 61
# speedup vs baseline: 1.5848x; 1.0519x over previous
"""Trainium2 Bass kernel for the Koopman-operator rollout.

Reference computation: y0 = x[:, 0, :]  (shape [2048, 256]);
    y_t = y_{t-1} @ W.T  for t = 1..512, Y[:, t-1, :] = y_t.
Output: [2048, 512, 256] fp32 (1 GiB).

Strategy (8 cores, data-parallel over batch, 256 rows/core):
  Let Wt = W.T.  Y[:, t] = y0 @ Wt^{t+1}.
  * Precompute P_j = Wt^j for j=1..16 via a log-depth product tree.
    Products use the duality Q_j = W^j = (P_j)^T so every product is
    expressible as matmul(out = lhsT.T @ rhs) with natural layouts.
  * Checkpoint states Z_i = y0 @ Wt^{16 i} (i=0..31), kept TRANSPOSED
    (k on partitions) so they can serve as matmul operands. Computed by
    prefix-doubling jumps A_m = Wt^{16 m} (m=1,2,4,8,16) -> rounding
    depth O(log T) instead of 512.
  * Per checkpoint i: Y[:, 16i+j-1] = Z_i @ P_j for j=1..16, as dense
    N=512 matmuls with Z_i^T stationary.
  Matmul-operand tiles are float32r (full PE rate at N>=256, reduced
  multiply precision, fp32 PSUM accumulation).

Performance structure (cost-model timeline ~254 us/core):
  * Output is stored to HBM as bf16 (64 MiB/core instead of 128 MiB),
    halving the dominant write traffic; the PSUM->SBUF copies cast
    fp32 -> bf16 and the host upconverts after gather. This moves the
    kernel from DMA-bound (~375 us write floor) to PE-bound (~243 us
    of matmul issue at full f32r rate).
  * Uniform software pipeline: each Z-jump is followed by the PREVIOUS
    checkpoint's reserved output groups (fill covering the jump's
    PSUM->SBUF copy latency), then the new checkpoint's main groups,
    so PE never waits on a fresh copy. The A-ladder products ride the
    same pipeline (anchors 2,4,8,16 are jumped straight from Z0).
  * PSUM->SBUF copies alternate DVE/ACT (1:1 for output tiles; per-
    product for the tree); DMAs are 0.75/0.25 MiB HWDGE chunks, the
    final checkpoint ships in 256 KiB chunks to shorten the drain.

  Measured rel err vs fp32 CPU oracle: 4.2e-3 (gate 2e-2).
"""

import os

import numpy as np

import concourse.bass as bass
import concourse.mybir as mybir
import concourse.tile as tile
from concourse import bacc
from concourse.bass import ds
from concourse.bass_utils import run_bass_kernel_spmd
from concourse.masks import make_identity

F32 = mybir.dt.float32
F32R = mybir.dt.float32r
BF16 = mybir.dt.bfloat16

# Output HBM format: bf16 halves the dominant HBM write traffic (128 MiB ->
# 64 MiB per core); the fp32 upconvert happens on the host after gather.
# Quantization adds ~1e-3 rel err, far under the 2e-2 gate.
OUT_DT = BF16

N_CORES = 8
B_FULL = 2048
B_SH = B_FULL // N_CORES  # 256 batch rows per core
K = 256  # state dim
T = 512  # time steps
S = 16  # timesteps per checkpoint chunk
M = T // S  # 32 checkpoints

# engine split for PSUM->SBUF output copies: of every K_COPY_MOD tiles,
# the first K_COPY_DVE go to VectorE (DVE), the rest to ScalarE (ACT).
# DVE also carries the P-tree/Z-jump copies, so it gets the minority share.
COPY_DVE = int(os.environ.get("K_COPY_DVE", "1"))
COPY_MOD = int(os.environ.get("K_COPY_MOD", "2"))


def _mm(nc, out, lhsT, rhs, start, stop):
    # operands are float32r tiles already (producers round to f32r)
    nc.tensor.matmul(out, lhsT, rhs, start=start, stop=stop)


class _Mat:
    """A 256x256 matrix stored as an SBUF tile [128, 2, 256]:
    elem (p, h, c) = M[h*128 + p, c]."""

    def __init__(self, ap):
        self.ap = ap

    def half(self, hm):
        # [128, 256] slice: rows hm*128 .. hm*128+127 (partition = row)
        return self.ap[:, hm, :]

    def blk(self, hm, hc):
        # [128, 128] block: rows hm*128.., cols hc*128..
        return self.ap[:, hm, ds(128 * hc, 128)]


_prod_ctr = [0]


def _product(nc, psum_pool, dst, lhsT_mat, rhs_mat):
    """dst = lhsT_mat.T @ rhs_mat  (all 256x256 _Mats).

    One full-bank PSUM tile + a single [128, 512] copy, alternating the
    copy engine per product so chained products don't serialize on DVE."""
    ps = psum_pool.tile([128, 2, 256], F32, tag="psz", name=f"psz_{_prod_ctr[0]}")
    for ha in range(2):
        for hm in range(2):
            _mm(nc, ps[:, ha, :], lhsT_mat.blk(hm, ha), rhs_mat.half(hm),
                hm == 0, hm == 1)
    if _prod_ctr[0] % 2 == 0:
        nc.vector.tensor_copy(dst.ap, ps)
    else:
        nc.scalar.copy(dst.ap, ps)
    _prod_ctr[0] += 1


def _build_program():
    nc = bacc.Bacc(
        "TRN2",
        target_bir_lowering=False,
        debug=False,
        enable_asserts=False,
        num_devices=N_CORES,
    )
    x_d = nc.dram_tensor("x", [B_SH, K], F32, kind="ExternalInput").ap()
    w_d = nc.dram_tensor("w", [K, K], F32, kind="ExternalInput").ap()
    y_d = nc.dram_tensor("y", [B_SH, T, K], OUT_DT, kind="ExternalOutput").ap()

    with tile.TileContext(nc) as tc:
        with (
            tc.tile_pool(name="consts", bufs=1) as consts,
            tc.tile_pool(name="mats", bufs=1) as mats,
            tc.tile_pool(name="zts", bufs=1) as zts,
            tc.tile_pool(name="ostage", bufs=int(os.environ.get("K_OST", "5"))) as ostage,
            tc.tile_pool(name="pso", bufs=int(os.environ.get("K_PSO", "5")), space="PSUM") as pso,
            tc.tile_pool(name="psz", bufs=int(os.environ.get("K_PSZ", "3")), space="PSUM") as psz,
        ):
            w_nat = consts.tile([128, 2, K], F32, tag="w_nat", name="w_nat")
            x_nat = consts.tile([128, 2, K], F32, tag="x_nat", name="x_nat")
            for h in range(2):
                nc.sync.dma_start(out=w_nat[:, h, :], in_=w_d[ds(128 * h, 128), :])
            nc.sync.dma_start(
                out=x_nat, in_=x_d.rearrange("(h p) k -> p h k", p=128)
            )

            ident = consts.tile([128, 128], F32, tag="ident", name="ident")
            make_identity(nc, ident)

            # Warm the PE clock ramp (HAM un-throttles after ~3us of
            # sustained activity) with identity self-transposes while the
            # W DMA is still in flight, so the real products run at 2.4 GHz.
            n_warm = int(os.environ.get("K_WARM", "10"))
            for wi in range(n_warm):
                jk = psz.tile([128, 2, 256], F32, tag="psz", name=f"warm_{wi}")
                nc.tensor.transpose(jk[:, 0, ds(0, 128)], ident, ident)

            # Pcat holds P_1..P_16 row-half-major: [128, 2, 16*256]
            pcat = mats.tile([128, 2, S * K], F32R, tag="pcat", name="pcat")

            def P(j):  # 1-indexed power as a _Mat-like view
                class V:
                    ap = pcat[:, :, ds(K * (j - 1), K)]

                    def half(self, hm, _j=j):
                        return pcat[:, hm, ds(K * (_j - 1), K)]

                    def blk(self, hm, hc, _j=j):
                        return pcat[:, hm, ds(K * (_j - 1) + 128 * hc, 128)]

                return V()

            w_r = consts.tile([128, 2, K], F32R, tag="w_r", name="w_r")
            for h in range(2):
                nc.vector.tensor_copy(w_r[:, h, :], w_nat[:, h, :])
            q1 = _Mat(w_r)  # Q_1 = W (natural layout, rounded to f32r)

            # --- transposes: Z0^T = x^T, P_1 = W^T (PE transpose via identity)
            zt = [None] * M
            zt[0] = _Mat(zts.tile([128, 2, K], F32R, tag="zt0", name="zt0"))
            p1 = P(1)
            # W-transposes first: P_1 gates the whole P-tree.
            for h in range(2):
                tpw = psz.tile([128, 2, 128], F32, tag="psz", name=f"pstw_{h}")
                for g in range(2):
                    nc.tensor.transpose(
                        tpw[:, g, :], w_nat[:, g, ds(128 * h, 128)], ident
                    )
                eng = nc.vector.tensor_copy if h == 0 else nc.scalar.copy
                eng(pcat[:, h, ds(0, 256)], tpw)

            def emit_x_transposes():
                for h in range(2):
                    tp = psz.tile(
                        [128, 2, 128], F32, tag="psz", name=f"pstx_{h}"
                    )
                    for g in range(2):
                        nc.tensor.transpose(
                            tp[:, g, :], x_nat[:, g, ds(128 * h, 128)], ident
                        )
                    eng = nc.vector.tensor_copy if h == 0 else nc.scalar.copy
                    eng(zt[0].ap[:, h, :], tp)

            # --- P-tree: P_1..P_16 (+ Q_2, Q_4, Q_8)
            def mk(tag):
                return _Mat(mats.tile([128, 2, K], F32R, tag=tag, name=tag))

            # --- checkpoint Z-tree (prefix doubling) interleaved with outputs
            copy_ctr = [0]

            def emit_outputs(i, ns=range(8), dma_split=1, ms=(0, 1)):
                """Y[:, 16i + j - 1, :] = Z_i @ P_j for j-pairs in ns
                (contiguous), staged through SBUF and shipped by dma_split
                DMAs per batch half."""
                ns = list(ns)
                nt = 2 * len(ns)  # timesteps staged
                for m in ms:  # batch half
                    ost = ostage.tile(
                        [128, nt, K], OUT_DT, tag=f"ost{nt}",
                        name=f"ost_{i}_{m}_{ns[0]}",
                    )
                    pos = {}
                    for n in ns:
                        pos[n] = pso.tile(
                            [128, 2, K], F32, tag="pso", name=f"pso_{i}_{m}_{n}"
                        )
                    for hm in range(2):
                        lhsT = zt[i].ap[:, hm, ds(128 * m, 128)]
                        for n in ns:
                            # rhs: P_{2n+1}, P_{2n+2} concatenated = 512 cols
                            rhs = pcat[:, hm, ds(512 * n, 512)]
                            _mm(nc, pos[n], lhsT, rhs, hm == 0, hm == 1)
                    per_dma = len(ns) // dma_split
                    for g in range(dma_split):
                        grp = ns[g * per_dma : (g + 1) * per_dma]
                        for n in grp:
                            dst = ost[:, ds(2 * (n - ns[0]), 2), :]
                            if copy_ctr[0] % COPY_MOD < COPY_DVE:
                                nc.vector.tensor_copy(dst, pos[n])
                            else:
                                nc.scalar.copy(dst, pos[n])
                            copy_ctr[0] += 1
                        n0 = grp[0]
                        nc.sync.dma_start(
                            out=y_d[
                                ds(128 * m, 128),
                                ds(S * i + 2 * n0, 2 * len(grp)),
                                :,
                            ],
                            in_=ost[
                                :, ds(2 * (n0 - ns[0]), 2 * len(grp)), :
                            ],
                        )

            def emit_zjump(dst_i, src_i, m):
                zt[dst_i] = _Mat(
                    zts.tile([128, 2, K], F32R, tag=f"zt{dst_i}", name=f"zt{dst_i}")
                )
                # Z_{dst}^T = A_m^T @ Z_{src}^T
                _product(nc, psz, zt[dst_i], amat[m], zt[src_i])

            q2, q4, q8 = mk("q2"), mk("q4"), mk("q8")
            _product(nc, psz, P(2), q1, p1)  # P2 = Q1.T @ P1 = Wt^2
            _product(nc, psz, q2, p1, q1)  # Q2 = P1.T @ Q1 = W^2
            # x-transposes here fill PE while P2/Q2's copies land
            emit_x_transposes()
            emit_outputs(0, ns=[0])  # needs P1, P2 + Z0^T only
            _product(nc, psz, P(3), q1, P(2))
            _product(nc, psz, P(4), q2, P(2))
            emit_outputs(0, ns=[1])  # needs P3, P4
            _product(nc, psz, q4, P(2), q2)
            for j in range(1, 5):
                _product(nc, psz, P(4 + j), q4, P(j))
            emit_outputs(0, ns=[2, 3])  # needs P5..P8
            _product(nc, psz, q8, P(4), q4)
            for j in range(1, 9):
                _product(nc, psz, P(8 + j), q8, P(j))

            # --- A-ladder interleaved with anchor jumps + output bursts.
            # A_m = Wt^{16 m}; each ladder product immediately enables the
            # anchor checkpoint 2^k, whose 32 output matmuls (6.8us) hide the
            # next serial ladder step (product+copy ~1.3us). Jump depth stays
            # logarithmic (zt[31] is 5 jumps from zt[0]).
            # --- A-ladder + checkpoint jumps + output bursts, as a uniform
            # software pipeline. Per iteration:
            #   zjump(tgt)  (4 matmuls)
            #   ladder products whose inputs landed >=1 iteration ago
            #   outputs(prev)[6,7]   <- 1.7us fill covering zjump's copy
            #   outputs(tgt)[0..5]   <- main burst (5.1us)
            # so no matmul ever waits on a fresh PSUM->SBUF copy.
            amat = {1: P(16)}
            q16 = mk("q16")
            _product(nc, psz, q16, P(8), q8)  # W^16
            emit_outputs(0, ns=[4])  # fill: q16 copy
            a2 = mk("a2")
            qlad_a = mk("qlad_a")  # W^32, later reused for W^128
            _product(nc, psz, a2, q16, amat[1])  # Wt^32
            _product(nc, psz, qlad_a, amat[1], q16)  # W^32
            amat[2] = a2
            emit_outputs(0, ns=[5])  # fill: a2/qlad_a copies

            def lad_a4():
                a4 = mk("a4")
                _product(nc, psz, a4, qlad_a, a2)  # Wt^64
                qlad_b = mk("qlad_b")
                _product(nc, psz, qlad_b, a2, qlad_a)  # W^64
                amat[4] = a4
                _lad.update(qlad_b=qlad_b)

            def lad_a8():
                a8 = mk("a8")
                _product(nc, psz, a8, _lad["qlad_b"], amat[4])  # Wt^128
                _product(nc, psz, qlad_a, amat[4], _lad["qlad_b"])  # W^128
                amat[8] = a8

            def lad_a16():
                a16 = mk("a16")
                _product(nc, psz, a16, qlad_a, amat[8])  # Wt^256
                amat[16] = a16

            _lad = {}
            jump_plan = (
                [(2, 0, 2, lad_a4), (4, 0, 4, lad_a8), (8, 0, 8, lad_a16),
                 (16, 0, 16, None), (24, 16, 8, None)]
                + [(src + 4, src, 4, None) for src in (8, 16, 24)]
                + [(src + 2, src, 2, None)
                   for src in (4, 8, 12, 16, 20, 24, 28)]
                + [(src + 1, src, 1, None) for src in range(0, 31, 2)]
            )
            prev = 0
            for tgt, src, m, prods in jump_plan:
                emit_zjump(tgt, src, m)
                if prods is not None:
                    prods()
                emit_outputs(prev, ns=[6, 7])
                if tgt == 31:
                    # last checkpoint: fine DMA splits so the post-compute
                    # drain is one 256 KiB chunk, not a full 1 MiB half.
                    emit_outputs(31, ms=(0,), dma_split=2)
                    emit_outputs(31, ms=(1,), dma_split=4)
                else:
                    emit_outputs(tgt, ns=[0, 1, 2, 3, 4, 5])
                prev = tgt

    nc.compile()
    return nc


_cached_nc = None
_last_results = None


def kernel(x, W, T=None):
    global _cached_nc, _last_results
    if _cached_nc is None:
        _cached_nc = _build_program()
    nc = _cached_nc

    x2 = np.ascontiguousarray(np.asarray(x, dtype=np.float32).reshape(B_FULL, K))
    w2 = np.ascontiguousarray(np.asarray(W, dtype=np.float32))
    in_maps = [
        {"x": x2[i * B_SH : (i + 1) * B_SH], "w": w2} for i in range(N_CORES)
    ]
    res = run_bass_kernel_spmd(
        nc,
        in_maps,
        core_ids=list(range(N_CORES)),
        trace=bool(os.environ.get("BASS_TRACE")),
    )
    _last_results = res
    y = np.concatenate(
        [np.asarray(res.results[i]["y"]).astype(np.float32) for i in range(N_CORES)],
        axis=0,
    )
    return y

